# revision 3
# baseline (speedup 1.0000x reference)
"""GAT (2-layer, PyG-style) on 8 Trainium2 NeuronCores.

Math
----
Layer 1 has in_channels=1, so h = x @ W1 is rank-1: every per-edge quantity
reduces to scalars per node.  With s1[h] = sum_c W1[h,c]*att_src1[h,c] and
d1[h] = sum_c W1[h,c]*att_dst1[h,c]:

    e[i,h]   = leaky_relu(s1[h]*x[src_i] + d1[h]*x[dst_i])
    denom[d,h] = sum_{i->d} exp(e[i,h])          (max-shift skipped: |e| < ~10)
    z[d,h]     = sum_{i->d} exp(e[i,h]) * x[src_i]
    out1[d,h,c] = elu(W1[h,c]*z/(denom+eps) + b1[h,c])

Layer 2 (heads=1, out=1) similarly only needs the scalar h2 = out1 @ W2.

Sharding: dst-owner node sharding (12500 nodes/core).  Per core, nodes are
degree-sorted and packed into 98 blocks of 128 (partition dim); each node's
incoming edges occupy W columns (block-group-padded).  x[dst] is then a free
per-partition broadcast and segment sums are row reductions; only x[src]
needs a real gather.

Gather (the whole ballgame)
---------------------------
Per-element indirect DMA costs ~360ns/descriptor (SWDGE desc-gen bound) --
80+ms for 3.4M edge-gathers.  Instead we use the MoE `dma_gather` custom
instruction (mlp ucode library), which on this HW sustains ~9ns per 256B
element on one SWDGE queue and ~2.5ns across 4 queues (1024 idxs per call;
larger calls crash).  The table is viewed as [1568, 64] f32 chunks (256B
rows, the instruction minimum); each edge slot fetches the 64-value chunk
CONTAINING its source scalar (idx = src//64, placed so idx position
c*128+p lands at grid slot (p, c)).  The wanted scalar is then selected by
an on-device one-hot mask, is_equal(off[p,c], iota64) with off=src%64
(255 for padding slots), times a free-axis add-reduce.  The h2 table for
layer 2 is the AllGather output in permuted node order, addressed via pos2
exactly like the baseline indirect version.  Self-loops (rank 0 of every
dst segment) are folded into denom/zt directly from x_own/h2_own, shrinking
every block's column width by 1 (C 1776 -> 1688).  Measured ~0.7ms per
216k-slot gather pass per core (~3.2ns/idx), ~2ms total vs the 85ms
indirect baseline.

(Walrus quirks: >2 sem waits per instruction are split onto injected NoOps;
drains likewise. tile's DMASW lane rotation must match the dma_gather
queue_num rotation, so NUM_SWDGE_GLOBAL_SEMS == NQ and dma_gather is the
only Pool-engine DMA in the program.)
"""
import sys
sys.path.insert(0, "/opt/trn_rl_repo")
import re
import numpy as np
import ml_dtypes
import concourse.bass as bass
import concourse.mybir as mybir
import concourse.tile as tile
from concourse.library_config import mlp
from concourse.library_overlay import lower_extended_insts
from concourse.bass_utils import run_bass_kernel_spmd
from bass_rust import ScopedClock, VectorClock

N = 100000
NCORES = 8
NPC = N // NCORES          # nodes per core
P = 128
NBLK = (NPC + P - 1) // P  # 98
ROWS = NBLK * P            # 12544
NEG_SLOPE = 0.2
EPS = 1e-16
BIG_NEG = -1.0e30
USE_ACT_LRELU = False
CHUNK = 64                 # gather element width (f32) = 256B
GC = 8                     # grid columns per dma_gather call (1024 idxs)
NQ = 4                     # SWDGE queues
NCHT = 1568                # table chunks = 100352/64
import os
REP_GATHER = int(os.environ.get("REP_GATHER", "1"))   # timing-slope knob
GSEL_BUFS = int(os.environ.get("GSEL_BUFS", "12"))    # in-flight gather tiles

F32 = mybir.dt.float32
BF16 = mybir.dt.bfloat16
I16 = mybir.dt.int16
AT = mybir.AluOpType
AF = mybir.ActivationFunctionType


# ---------------------------------------------------------------------------
# Tile tail-drain workaround: walrus TPB_CTRL codegen rejects a Drain with
# more than two sem waits; emit one NOP-wait per proc first.
def _split_drain_and_barrier(self, tick_clock, wait_clock):
    gc = tick_clock.global_clock
    ticks = [int(x) for x in re.findall(r"\d+", repr(gc))]
    for i, t in enumerate(ticks):
        if t <= 0:
            continue
        sub = VectorClock()
        sub.require_at_least(i, t)
        inst = self.nc.sync.nop()
        wait_clock.add_sem_waits(inst.ins, ScopedClock({None: sub}))
    self.nc.sync.drain()
    self.nc.all_engine_barrier()
    popped = self.nc._tile_sem_poison_stack.pop()
    assert popped is self._sem_poison
    self.nc.clear_and_free_semaphores(list(self.sems.allocated().values()))
    self.nc.all_engine_barrier()


tile.TileContext._drain_and_barrier = _split_drain_and_barrier

import concourse.tile_sem_assignment as _tsa
_tsa.NUM_SWDGE_GLOBAL_SEMS = NQ
_tsa.NUM_HWDGE_SEMS = 1

# Walrus encodes at most ~2 sem waits per instruction; split any excess onto
# injected same-engine NoOps in the BIR JSON right before compilation.
import json as _json
from concourse import bass2jax as _b2j


def _split_waits_json(bir, max_keep=1):
    d = _json.loads(bir)
    ctr = [0]

    def fix_block(blk):
        out = []
        for inst in blk.get("instructions", []):
            si = inst.get("sync_info")
            waits = (si or {}).get("on_wait") or []
            if len(waits) > max_keep and inst.get("opcode") != "NoOp":
                keep = waits[-max_keep:]
                for w in waits[:-max_keep]:
                    ctr[0] += 1
                    out.append({"debug": inst.get("debug", 0),
                                "engine": inst["engine"], "ins": [],
                                "outs": [], "name": f"I-wsp{ctr[0]}",
                                "opcode": "NoOp",
                                "sync_info": {"on_update": [], "on_wait": [w]}})
                si["on_wait"] = keep
            out.append(inst)
        blk["instructions"] = out
        for sb in blk.get("blocks", []):
            fix_block(sb)

    for fn in d["functions"]:
        for blk in fn["blocks"]:
            fix_block(blk)
    return _json.dumps(d).encode()


if not getattr(_b2j, "_wsplit_patched", False):
    _orig_cbk = _b2j.compile_bir_kernel

    def _cbk(bir, *a, **k):
        return _orig_cbk(_split_waits_json(bir), *a, **k)

    _b2j.compile_bir_kernel = _cbk
    _b2j._wsplit_patched = True


# ---------------------------------------------------------------------------
# CPU-side structural prep (graph topology only, no float math)

def _prep(edge_index):
    src = np.asarray(edge_index[0], dtype=np.int64)
    dst = np.asarray(edge_index[1], dtype=np.int64)
    loop = np.arange(N, dtype=np.int64)
    src = np.concatenate([loop, src])
    dst = np.concatenate([loop, dst])

    deg = np.bincount(dst, minlength=N)

    perms = []        # per core: processing order (local node ids 0..NPC-1)
    blk_deg = np.zeros((NCORES, NBLK), dtype=np.int64)
    for k in range(NCORES):
        dk = deg[k * NPC:(k + 1) * NPC]
        order = np.argsort(-dk, kind="stable").astype(np.int64)
        perms.append(order)
        dks = np.concatenate([dk[order], np.zeros(ROWS - NPC, np.int64)])
        blk_deg[k] = dks.reshape(NBLK, P).max(1)

    # Uniform group structure across cores: W per block = max over cores,
    # then greedily merge consecutive blocks (pad to group max) keeping the
    # added padding under ~8% and the per-group volume bounded.
    # Self-loops (rank 0 of every node, value available on-core) are folded
    # into the segment sums directly, so the grid only holds ranks >= 1.
    wblk = blk_deg.max(0) - 1      # [NBLK], non-increasing
    groups = []                    # list of (start_blk, end_blk, W)
    g0 = 0
    waste = 0.0
    real = 1.0
    for b in range(1, NBLK + 1):
        merge = False
        if b < NBLK:
            new_waste = waste + (wblk[g0] - wblk[b])
            new_real = real + wblk[b]
            vol = (b + 1 - g0) * max(wblk[g0], 1)
            if new_waste <= 0.08 * new_real and vol <= 256:
                merge = True
        if merge:
            waste, real = new_waste, new_real
            continue
        groups.append((g0, b, int(max(wblk[g0], 1))))
        if b < NBLK:
            g0 = b
            waste = 0.0
            real = float(wblk[b])
    col_off = np.zeros(NBLK, dtype=np.int64)
    C = 0
    for (a, b, w) in groups:
        for blk in range(a, b):
            col_off[blk] = C
            C += w
    if C % GC:
        C += GC - C % GC        # gather-call granularity

    # permuted-global position of each node (for the h2 gather)
    pos2 = np.zeros(N, dtype=np.int64)
    for k in range(NCORES):
        inv = np.zeros(NPC, dtype=np.int64)
        inv[perms[k]] = np.arange(NPC)
        pos2[k * NPC:(k + 1) * NPC] = k * ROWS + inv

    pos_a = np.zeros((NCORES, P, C), dtype=np.int64)   # layer-1 gather pos
    pos_b = np.zeros((NCORES, P, C), dtype=np.int64)   # layer-2 gather pos
    valid = np.zeros((NCORES, P, C), dtype=bool)

    order = np.argsort(dst, kind="stable")
    src_s = src[order]
    dst_s = dst[order]
    starts = np.searchsorted(dst_s, np.arange(N + 1))

    for k in range(NCORES):
        inv_k = np.zeros(NPC, dtype=np.int64)
        inv_k[perms[k]] = np.arange(NPC)
        base = k * NPC
        lo, hi = starts[base], starts[base + NPC]
        d_loc = dst_s[lo:hi] - base                  # local dst id
        i_proc = inv_k[d_loc]                        # processing index
        blk = i_proc // P
        p = i_proc - blk * P
        seg_start = starts[d_loc + base] - lo        # rank within segment
        rank = np.arange(hi - lo) - seg_start
        nsl = rank >= 1                              # rank 0 = self-loop
        c = col_off[blk[nsl]] + rank[nsl] - 1
        p = p[nsl]
        pos_a[k, p, c] = src_s[lo:hi][nsl]
        pos_b[k, p, c] = pos2[src_s[lo:hi][nsl]]
        valid[k, p, c] = True

    maskneg = np.where(valid, 0.0, BIG_NEG).astype(np.float32)

    return {
        "groups": groups, "C": C, "perms": perms,
        "pos_a": pos_a, "pos_b": pos_b, "valid": valid,
        "maskneg": maskneg,
    }


def _mk_gather_inputs(pos, valid, C):
    """idx tile [16, ncalls*64] int16 + within-chunk offsets [P, C] bf16
    (invalid slots get 255 so the on-device iota-eq mask comes out 0)."""
    ncalls = C // GC
    chunk = (pos // CHUNK).astype(np.int16)          # [P, C]
    off = np.where(valid, pos % CHUNK, 255).astype(np.float32)
    arr = chunk.reshape(P, ncalls, GC).transpose(1, 2, 0).reshape(
        ncalls, GC * P)                              # [g, j] j=c_local*128+p
    w = arr.reshape(ncalls, (GC * P) // 16, 16).transpose(0, 2, 1)
    t16 = np.ascontiguousarray(
        w.transpose(1, 0, 2).reshape(16, ncalls * 64).astype(np.int16))
    return t16, np.ascontiguousarray(off.astype(ml_dtypes.bfloat16))


# ---------------------------------------------------------------------------
# Bass program (identical for all cores; per-core data differs)

def _build(C, groups):
    nc = bass.Bass("TRN2", target_bir_lowering=False, debug=False,
                   num_devices=NCORES, num_swdge_queues=NQ,
                   dynamic_dma_scratch_size=65536)
    ncalls = C // GC
    xpad = nc.dram_tensor("xpad", [NCHT, CHUNK], F32,
                          kind="ExternalInput").ap()
    x_own = nc.dram_tensor("x_own", [P, NBLK], F32, kind="ExternalInput").ap()
    idxa = nc.dram_tensor("idxa", [16, ncalls * 64], I16,
                          kind="ExternalInput").ap()
    idxb = nc.dram_tensor("idxb", [16, ncalls * 64], I16,
                          kind="ExternalInput").ap()
    ma = nc.dram_tensor("offa", [P, C], BF16, kind="ExternalInput").ap()
    mb = nc.dram_tensor("offb", [P, C], BF16, kind="ExternalInput").ap()
    iota64 = nc.dram_tensor("iota64", [1, 64], F32, kind="ExternalInput").ap()
    mneg = nc.dram_tensor("mneg", [P, C], F32, kind="ExternalInput").ap()
    w1 = nc.dram_tensor("w1", [1, 128], F32, kind="ExternalInput").ap()
    as1 = nc.dram_tensor("as1", [1, 128], F32, kind="ExternalInput").ap()
    ad1 = nc.dram_tensor("ad1", [1, 128], F32, kind="ExternalInput").ap()
    b1 = nc.dram_tensor("b1", [1, 128], F32, kind="ExternalInput").ap()
    w2 = nc.dram_tensor("w2", [1, 128], F32, kind="ExternalInput").ap()
    sc2 = nc.dram_tensor("sc2", [1, 8], F32, kind="ExternalInput").ap()
    # sc2 row: [att_src2, att_dst2, b2, 0 | iota4]

    out_d = nc.dram_tensor("out", [P, NBLK], F32, kind="ExternalOutput").ap()
    h2t_slice = nc.dram_tensor("h2t_slice", [ROWS // CHUNK, CHUNK], F32,
                               kind="Internal").ap()
    h2t_full = nc.dram_tensor("h2t_full", [NCHT, CHUNK], F32,
                              kind="Internal", addr_space="Shared").ap()

    with tile.TileContext(nc, num_cores=NCORES) as tc:
        _body(nc, tc, C, groups, xpad, x_own, idxa, idxb, ma, mb, iota64,
              mneg, w1, as1, ad1, b1, w2, sc2, out_d, h2t_slice, h2t_full)
    lower_extended_insts(nc)
    return nc


def _gather_v2(nc, tc, C, table_d, idx_t, off_t, iota_t, xs, nreg, qctr):
    """xs[p, c] = table.flat[pos[p, c]] via chunked dma_gather + mask-select.

    Per call: 1024 chunk-indices -> [128, GC, 64] f32; the one-hot select
    mask is built on-device as is_equal(off[p,c], iota[k]) (invalid slots
    carry off=255 so they select nothing), then mask-mult + free-axis add
    reduce pick out the wanted scalar per slot."""
    ncalls = C // GC
    with tc.tile_pool(name="gsel", bufs=GSEL_BUFS) as gp:
        for g in range(ncalls):
            ch = gp.tile([P, GC * CHUNK], F32, tag="ch")
            ch3 = ch[:].rearrange("p (g k) -> p g k", k=CHUNK)
            nc.gpsimd.dma_gather(
                ch3, table_d, idx_t[:, g * 64:(g + 1) * 64],
                GC * P, nreg, CHUNK, queue_num=qctr[0] % NQ)
            qctr[0] += 1
            m = gp.tile([P, GC * CHUNK], F32, tag="m")
            nc.vector.tensor_tensor(
                out=m[:].rearrange("p (g k) -> p g k", k=CHUNK),
                in0=off_t[:, g * GC:(g + 1) * GC].rearrange(
                    "p g -> p g ()").to_broadcast([P, GC, CHUNK]),
                in1=iota_t.rearrange("p k -> p () k").to_broadcast(
                    [P, GC, CHUNK]),
                op=AT.is_equal)
            nc.vector.tensor_tensor(out=ch[:], in0=ch[:], in1=m[:],
                                    op=AT.mult)
            nc.vector.tensor_reduce(
                out=xs[:, g * GC:(g + 1) * GC],
                in_=ch3, axis=mybir.AxisListType.X, op=AT.add)


def _body(nc, tc, C, groups, xpad_d, x_own_d, idxa_d, idxb_d, ma_d, mb_d,
          iota64_d, mneg_d, w1_d, as1_d, ad1_d, b1_d, w2_d, sc2_d,
          out_d, h2t_slice, h2t_full):
    import contextlib
    ctx = contextlib.ExitStack()
    H = 8
    ncalls = C // GC
    with ctx:
        nc.gpsimd.load_library(mlp)
        nreg = nc.gpsimd.to_reg(GC * P)
        qctr = [0]
        const = ctx.enter_context(tc.tile_pool(name="const", bufs=1))
        group_c0 = {}
        _c = 0
        for (ga, gb, gw) in groups:
            group_c0[ga] = _c
            _c += gw * (gb - ga)

        # ---- persistent loads
        mneg = const.tile([P, C], F32)
        nc.sync.dma_start(mneg[:], mneg_d[:])
        idx_t = const.tile([P, ncalls * 64], I16)
        for k in range(8):
            nc.sync.dma_start(idx_t[16 * k:16 * (k + 1), :], idxa_d[:])
        off_t = const.tile([P, C], BF16)
        nc.sync.dma_start(off_t[:], ma_d[:])
        x_own = const.tile([P, NBLK], F32)
        nc.sync.dma_start(x_own[:], x_own_d[:])

        # ---- params: one row, then broadcast via ones-matmul
        # 0:128 w1 | 128:256 as1 | 256:384 ad1 | 384:512 b1 | 512:640 w2
        # 640:648 sc2 (att_src2, att_dst2, b2, w2sum | iota4)
        # 648:656 s1 | 656:664 d1 | 664:728 iota64
        prow = const.tile([1, 728], F32)
        nc.sync.dma_start(prow[:, 0:128], w1_d[:])
        nc.sync.dma_start(prow[:, 128:256], as1_d[:])
        nc.sync.dma_start(prow[:, 256:384], ad1_d[:])
        nc.sync.dma_start(prow[:, 384:512], b1_d[:])
        nc.sync.dma_start(prow[:, 512:640], w2_d[:])
        nc.sync.dma_start(prow[:, 640:648], sc2_d[:])
        nc.sync.dma_start(prow[:, 664:728], iota64_d[:])
        tmp = const.tile([1, 256], F32)
        nc.vector.tensor_tensor(out=tmp[:, 0:128], in0=prow[:, 0:128],
                                in1=prow[:, 128:256], op=AT.mult)
        nc.vector.tensor_tensor(out=tmp[:, 128:256], in0=prow[:, 0:128],
                                in1=prow[:, 256:384], op=AT.mult)
        nc.vector.tensor_reduce(out=prow[:, 648:664],
                                in_=tmp[:].rearrange("a (h c) -> a h c", c=16),
                                axis=mybir.AxisListType.X, op=AT.add)
        nc.vector.tensor_reduce(out=prow[:, 643:644], in_=prow[:, 512:640],
                                axis=mybir.AxisListType.X, op=AT.add)

        ones = const.tile([1, P], F32)
        nc.vector.memset(ones[:], 1.0)
        # funnel prow through one DVE copy so the matmul (whose load-weights
        # encoding has a tight sem-wait budget) depends on a single producer
        prow2 = const.tile([1, 728], F32)
        nc.vector.tensor_copy(out=prow2[:], in_=prow[:])
        psum = ctx.enter_context(tc.tile_pool(name="psum", bufs=2,
                                              space="PSUM"))
        pc = const.tile([P, 728], F32)
        for lo, hi in ((0, 512), (512, 728)):
            pcast = psum.tile([P, 512], F32, tag="pcast")
            nc.tensor.matmul(pcast[:, :hi - lo], lhsT=ones[:],
                             rhs=prow2[:, lo:hi], start=True, stop=True)
            nc.vector.tensor_copy(out=pc[:, lo:hi], in_=pcast[:, :hi - lo])
        W1t = pc[:, 0:128]
        B1t = pc[:, 384:512]
        W2t = pc[:, 512:640]
        s2c = pc[:, 640:641]
        d2c = pc[:, 641:642]
        b2c = pc[:, 642:643]
        w2sum = pc[:, 643:644]
        s1c = pc[:, 648:656]
        d1c = pc[:, 656:664]
        iota_t = pc[:, 664:728]

        # ---- gather x[src] (layer 1)
        xs = const.tile([P, C], F32)
        for _ in range(REP_GATHER):
            _gather_v2(nc, tc, C, xpad_d, idx_t, off_t, iota_t, xs,
                       nreg, qctr)

        # adst[p, b, h] = x_own[p, b] * d1[h]
        adst = const.tile([P, NBLK * H], F32)
        nc.vector.tensor_tensor(
            out=adst[:].rearrange("p (b h) -> p b h", h=H),
            in0=x_own[:].rearrange("p b -> p b ()").to_broadcast([P, NBLK, H]),
            in1=d1c.rearrange("p h -> p () h").to_broadcast([P, NBLK, H]),
            op=AT.mult)

        denom = const.tile([P, NBLK * H], F32)
        zt = const.tile([P, NBLK * H], F32)

        # ---- layer-1 main, one run per block-group
        with tc.tile_pool(name="work", bufs=2) as work:
            _layer1_main(nc, C, groups, group_c0, work, xs, mneg, adst, s1c,
                         denom, zt)

        # ---- fold the self-loop edge (src == dst) into denom/zt directly:
        # e_self = lrelu((s1+d1)*x_own); denom += exp(e_self);
        # zt += exp(e_self)*x_own.  (Pad rows add exp(0)=1 to dead nodes.)
        sl1 = const.tile([P, H], F32)
        nc.vector.tensor_tensor(out=sl1[:], in0=s1c, in1=d1c, op=AT.add)
        us = const.tile([P, NBLK * H], F32)
        nc.vector.tensor_tensor(
            out=us[:].rearrange("p (b h) -> p b h", h=H),
            in0=x_own[:].rearrange("p b -> p b ()").to_broadcast([P, NBLK, H]),
            in1=sl1[:].rearrange("p h -> p () h").to_broadcast([P, NBLK, H]),
            op=AT.mult)
        nc.vector.scalar_tensor_tensor(out=us[:], in0=us[:], scalar=NEG_SLOPE,
                                       in1=us[:], op0=AT.mult, op1=AT.max)
        us2 = const.tile([P, NBLK * H], F32)
        nc.scalar.activation(out=us2[:], in_=us[:], func=AF.Exp)
        nc.vector.tensor_tensor(out=denom[:], in0=denom[:], in1=us2[:],
                                op=AT.add)
        nc.vector.tensor_tensor(
            out=us2[:].rearrange("p (b h) -> p b h", h=H),
            in0=us2[:].rearrange("p (b h) -> p b h", h=H),
            in1=x_own[:].rearrange("p b -> p b ()").to_broadcast([P, NBLK, H]),
            op=AT.mult)
        nc.vector.tensor_tensor(out=zt[:], in0=zt[:], in1=us2[:], op=AT.add)

        # ---- layer-1 epilogue -> h2_own [P, NBLK]
        r = const.tile([P, NBLK * H], F32)
        nc.vector.tensor_scalar(out=r[:], in0=denom[:], scalar1=float(EPS),
                                scalar2=None, op0=AT.add)
        nc.vector.reciprocal(out=r[:], in_=r[:])
        nc.vector.tensor_tensor(out=r[:], in0=r[:], in1=zt[:], op=AT.mult)

        h2_own = const.tile([P, NBLK], F32)
        with tc.tile_pool(name="ep", bufs=2) as ep:
            _epilogue(nc, ep, r, W1t, B1t, W2t, h2_own)
        nc.vector.tensor_scalar(out=h2_own[:], in0=h2_own[:], scalar1=w2sum,
                                scalar2=None, op0=AT.subtract)
        _rest(nc, tc, C, groups, group_c0, const, mneg, h2_own, idx_t,
              idxb_d, mb_d, off_t, iota_t, s2c, d2c, b2c, out_d,
              h2t_slice, h2t_full, nreg, qctr)


def _layer1_main(nc, C, groups, group_c0, work, xs, mneg, adst, s1c,
                 denom, zt):
        H = 8
        for (a, b, w) in groups:
            nb = b - a
            c0 = group_c0[a]
            V = nb * H * w
            xs_g = xs[:, c0:c0 + nb * w].rearrange("p (n w) -> p n () w", w=w)
            mn_g = mneg[:, c0:c0 + nb * w].rearrange("p (n w) -> p n () w", w=w)
            ad_g = adst[:, a * H:b * H].rearrange("p (n h) -> p n h ()", h=H)
            s1_g = s1c.rearrange("p h -> p () h ()")

            u = work.tile([P, V], F32, tag="u")
            u4 = u[:].rearrange("p (n h w) -> p n h w", h=H, w=w)
            nc.vector.tensor_tensor(out=u4, in0=xs_g.to_broadcast([P, nb, H, w]),
                                    in1=s1_g.to_broadcast([P, nb, H, w]), op=AT.mult)
            u2 = work.tile([P, V], F32, tag="u2")
            u24 = u2[:].rearrange("p (n h w) -> p n h w", h=H, w=w)
            nc.vector.tensor_tensor(out=u24, in0=u4,
                                    in1=ad_g.to_broadcast([P, nb, H, w]), op=AT.add)
            nc.vector.tensor_tensor(out=u4, in0=u24,
                                    in1=mn_g.to_broadcast([P, nb, H, w]), op=AT.add)
            # leaky relu: max(0.2*v, v), then exp
            if USE_ACT_LRELU:
                nc.scalar.activation(out=u24, in_=u4, func=AF.Lrelu,
                                     alpha=NEG_SLOPE)
            else:
                nc.vector.scalar_tensor_tensor(out=u24, in0=u4, scalar=NEG_SLOPE,
                                               in1=u4, op0=AT.mult, op1=AT.max)
            ex = work.tile([P, V], F32, tag="ex")
            ex4 = ex[:].rearrange("p (n h w) -> p n h w", h=H, w=w)
            nc.scalar.activation(out=ex4, in_=u24, func=AF.Exp)
            nc.vector.tensor_reduce(
                out=denom[:, a * H:b * H].rearrange("p (n h) -> p n h", h=H),
                in_=ex4, axis=mybir.AxisListType.X, op=AT.add)
            nc.vector.tensor_tensor(out=u4, in0=ex4,
                                    in1=xs_g.to_broadcast([P, nb, H, w]), op=AT.mult)
            nc.vector.tensor_reduce(
                out=zt[:, a * H:b * H].rearrange("p (n h) -> p n h", h=H),
                in_=u4, axis=mybir.AxisListType.X, op=AT.add)

def _epilogue(nc, ep, r, W1t, B1t, W2t, h2_own):
        H = 8
        EPB = 14
        for a in range(0, NBLK, EPB):
            b = min(a + EPB, NBLK)
            nb = b - a
            V = nb * 128
            v = ep.tile([P, EPB * 128], F32, tag="v")
            v4 = v[:, :V].rearrange("p (n h c) -> p n h c", h=H, c=16)
            r_g = r[:, a * H:b * H].rearrange("p (n h) -> p n h ()", h=H)
            w1_g = W1t.rearrange("p (h c) -> p () h c", c=16)
            b1_g = B1t.rearrange("p (h c) -> p () h c", c=16)
            nc.vector.tensor_tensor(out=v4, in0=r_g.to_broadcast([P, nb, H, 16]),
                                    in1=w1_g.to_broadcast([P, nb, H, 16]),
                                    op=AT.mult)
            v2 = ep.tile([P, EPB * 128], F32, tag="v2")
            nc.vector.tensor_tensor(
                out=v2[:, :V].rearrange("p (n h c) -> p n h c", h=H, c=16),
                in0=v4, in1=b1_g.to_broadcast([P, nb, H, 16]), op=AT.add)
            # h1' = max(v,0) + min(exp(v),1);  elu(v) = h1' - 1
            ev = ep.tile([P, EPB * 128], F32, tag="ev")
            nc.scalar.activation(out=ev[:, :V], in_=v2[:, :V], func=AF.Exp)
            nc.vector.tensor_scalar(out=ev[:, :V], in0=ev[:, :V], scalar1=1.0,
                                    scalar2=None, op0=AT.min)
            nc.vector.tensor_scalar(out=v2[:, :V], in0=v2[:, :V], scalar1=0.0,
                                    scalar2=None, op0=AT.max)
            nc.vector.tensor_tensor(out=v2[:, :V], in0=v2[:, :V], in1=ev[:, :V],
                                    op=AT.add)
            # h2 = sum h1'*W2 - W2sum  (the elu -1 folded into W2sum)
            w2_g = W2t.rearrange("p (h c) -> p () (h c)", c=16)
            nc.vector.tensor_tensor(
                out=v4, in0=v2[:, :V].rearrange("p (n f) -> p n f", f=128),
                in1=w2_g.to_broadcast([P, nb, 128]), op=AT.mult)
            nc.vector.tensor_reduce(
                out=h2_own[:, a:b], in_=v4.rearrange("p n h c -> p n (h c)"),
                axis=mybir.AxisListType.X, op=AT.add)
def _rest(nc, tc, C, groups, group_c0, const, mneg, h2_own, idx_t,
          idxb_d, mb_d, off_t, iota_t, s2c, d2c, b2c, out_d,
          h2t_slice, h2t_full, nreg, qctr):
        # ---- store own h2 slice, AllGather the table
        nc.sync.dma_start(
            h2t_slice[:].rearrange("r c -> (r c)").rearrange(
                "(b p) -> p b", p=P),
            h2_own[:])
        nc.gpsimd.collective_compute(
            "AllGather", AT.bypass,
            replica_groups=[list(range(NCORES))],
            ins=[h2t_slice[:]], outs=[h2t_full[:]])

        # ---- layer 2: reuse the idx/off tiles' SBUF for idxb/offb
        for k in range(8):
            nc.sync.dma_start(idx_t[16 * k:16 * (k + 1), :], idxb_d[:])
        nc.sync.dma_start(off_t[:], mb_d[:])
        h2s = const.tile([P, C], F32)
        for _ in range(REP_GATHER):
            _gather_v2(nc, tc, C, h2t_full, idx_t, off_t, iota_t, h2s,
                       nreg, qctr)

        adst2 = const.tile([P, NBLK], F32)
        nc.vector.tensor_scalar(out=adst2[:], in0=h2_own[:], scalar1=d2c,
                                scalar2=None, op0=AT.mult)

        den2 = const.tile([P, NBLK], F32)
        z2 = const.tile([P, NBLK], F32)
        with tc.tile_pool(name="work2", bufs=2) as work:
            _layer2_main(nc, groups, group_c0, work, h2s, mneg, adst2, s2c,
                         den2, z2)

        # ---- self-loop fold, layer 2 (scalar per node)
        sl2 = const.tile([P, 1], F32)
        nc.vector.tensor_tensor(out=sl2[:], in0=s2c, in1=d2c, op=AT.add)
        u2s = const.tile([P, NBLK], F32)
        nc.vector.tensor_scalar(out=u2s[:], in0=h2_own[:], scalar1=sl2,
                                scalar2=None, op0=AT.mult)
        nc.vector.scalar_tensor_tensor(out=u2s[:], in0=u2s[:],
                                       scalar=NEG_SLOPE, in1=u2s[:],
                                       op0=AT.mult, op1=AT.max)
        u2e = const.tile([P, NBLK], F32)
        nc.scalar.activation(out=u2e[:], in_=u2s[:], func=AF.Exp)
        nc.vector.tensor_tensor(out=den2[:], in0=den2[:], in1=u2e[:],
                                op=AT.add)
        nc.vector.tensor_tensor(out=u2e[:], in0=u2e[:], in1=h2_own[:],
                                op=AT.mult)
        nc.vector.tensor_tensor(out=z2[:], in0=z2[:], in1=u2e[:], op=AT.add)
        _output(nc, den2, z2, b2c, out_d)


def _layer2_main(nc, groups, group_c0, work, h2s, mneg, adst2, s2c, den2, z2):
        for (a, b, w) in groups:
            nb = b - a
            c0 = group_c0[a]
            V = nb * w
            sl = slice(c0, c0 + V)
            h2s_g = h2s[:, sl].rearrange("p (n w) -> p n w", w=w)
            u = work.tile([P, V], F32, tag="u")
            u3 = u[:].rearrange("p (n w) -> p n w", w=w)
            nc.vector.scalar_tensor_tensor(
                out=u3, in0=h2s_g, scalar=s2c,
                in1=adst2[:, a:b].rearrange("p n -> p n ()").to_broadcast(
                    [P, nb, w]),
                op0=AT.mult, op1=AT.add)
            u2 = work.tile([P, V], F32, tag="u2")
            u23 = u2[:].rearrange("p (n w) -> p n w", w=w)
            nc.vector.tensor_tensor(
                out=u23, in0=u3,
                in1=mneg[:, sl].rearrange("p (n w) -> p n w", w=w), op=AT.add)
            if USE_ACT_LRELU:
                nc.scalar.activation(out=u3, in_=u23, func=AF.Lrelu,
                                     alpha=NEG_SLOPE)
            else:
                nc.vector.scalar_tensor_tensor(out=u3, in0=u23, scalar=NEG_SLOPE,
                                               in1=u23, op0=AT.mult, op1=AT.max)
            nc.scalar.activation(out=u23, in_=u3, func=AF.Exp)
            nc.vector.tensor_reduce(out=den2[:, a:b], in_=u23,
                                    axis=mybir.AxisListType.X, op=AT.add)
            nc.vector.tensor_tensor(out=u3, in0=u23, in1=h2s_g, op=AT.mult)
            nc.vector.tensor_reduce(out=z2[:, a:b], in_=u3,
                                    axis=mybir.AxisListType.X, op=AT.add)


def _output(nc, den2, z2, b2c, out_d):
        # ---- output
        nc.vector.tensor_scalar(out=den2[:], in0=den2[:], scalar1=float(EPS),
                                scalar2=None, op0=AT.add)
        nc.vector.reciprocal(out=den2[:], in_=den2[:])
        nc.vector.tensor_tensor(out=z2[:], in0=z2[:], in1=den2[:], op=AT.mult)
        nc.vector.tensor_scalar(out=z2[:], in0=z2[:], scalar1=b2c,
                                scalar2=None, op0=AT.add)
        nc.sync.dma_start(out_d[:], z2[:])


# ---------------------------------------------------------------------------

def kernel(**inputs):
    edge_index = np.asarray(inputs["edge_index"])
    prep = _prep(edge_index)
    C, groups, perms = prep["C"], prep["groups"], prep["perms"]

    x = np.asarray(inputs["x"], dtype=np.float32).reshape(-1)   # [N]
    xpad = np.ascontiguousarray(
        np.concatenate([x, np.zeros(NCHT * CHUNK - N, np.float32)]
                       ).reshape(NCHT, CHUNK))

    flat = lambda a: np.ascontiguousarray(
        np.asarray(a, dtype=np.float32).reshape(1, -1))
    w1 = flat(inputs["W1"]); as1 = flat(inputs["att_src1"])
    ad1 = flat(inputs["att_dst1"]); b1 = flat(inputs["b1"])
    w2 = flat(inputs["W2"])
    sc2 = np.zeros((1, 8), np.float32)
    sc2[0, 0] = np.asarray(inputs["att_src2"]).reshape(-1)[0]
    sc2[0, 1] = np.asarray(inputs["att_dst2"]).reshape(-1)[0]
    sc2[0, 2] = np.asarray(inputs["b2"]).reshape(-1)[0]
    sc2[0, 4:8] = [0.0, 1.0, 2.0, 3.0]
    iota64 = np.arange(64, dtype=np.float32).reshape(1, 64)

    nc = _build(C, groups)

    in_maps = []
    for k in range(NCORES):
        xk = x[k * NPC:(k + 1) * NPC][perms[k]]
        xk = np.concatenate([xk, np.zeros(ROWS - NPC, np.float32)])
        x_own = np.ascontiguousarray(xk.reshape(NBLK, P).T)
        ia, offa = _mk_gather_inputs(prep["pos_a"][k], prep["valid"][k], C)
        ib, offb = _mk_gather_inputs(prep["pos_b"][k], prep["valid"][k], C)
        in_maps.append({
            "xpad": xpad, "x_own": x_own,
            "idxa": ia, "idxb": ib, "offa": offa, "offb": offb,
            "iota64": iota64, "mneg": prep["maskneg"][k],
            "w1": w1, "as1": as1, "ad1": ad1, "b1": b1, "w2": w2, "sc2": sc2,
        })

    res = run_bass_kernel_spmd(nc, in_maps, core_ids=list(range(NCORES)))

    out = np.zeros((N, 1), np.float32)
    for k in range(NCORES):
        o = res.results[k]["out"]                    # [P, NBLK]
        flat_o = o.T.reshape(-1)[:NPC]
        out[k * NPC:(k + 1) * NPC, 0][perms[k]] = flat_o
    return out



# revision 13
# speedup vs baseline: 2.5445x; 2.5445x over previous
"""GAT (2-layer, PyG-style) on 8 Trainium2 NeuronCores.

Math
----
Layer 1 has in_channels=1, so h = x @ W1 is rank-1: every per-edge quantity
reduces to scalars per node.  With s1[h] = sum_c W1[h,c]*att_src1[h,c] and
d1[h] = sum_c W1[h,c]*att_dst1[h,c]:

    e[i,h]   = leaky_relu(s1[h]*x[src_i] + d1[h]*x[dst_i])
    denom[d,h] = sum_{i->d} exp(e[i,h])          (max-shift skipped: |e| < ~10)
    z[d,h]     = sum_{i->d} exp(e[i,h]) * x[src_i]
    out1[d,h,c] = elu(W1[h,c]*z/(denom+eps) + b1[h,c])

Layer 2 (heads=1, out=1) similarly only needs the scalar h2 = out1 @ W2.

Sharding: dst-owner node sharding (12500 nodes/core).  Per core, nodes are
degree-sorted and packed into 98 blocks of 128 (partition dim); each node's
incoming edges occupy W columns (block-group-padded).  x[dst] is then a free
per-partition broadcast and segment sums are row reductions; only x[src]
needs a real gather.

Gather (the whole ballgame)
---------------------------
Per-element indirect DMA costs ~360ns/descriptor (SWDGE desc-gen bound) --
80+ms for 3.4M edge-gathers.  Instead we use the MoE `dma_gather` custom
instruction (mlp ucode library; 1024 idxs per call, larger calls crash).
The table is viewed as [1568, 64] f32 chunks (256B rows, the instruction
minimum); each edge slot fetches the 64-value chunk CONTAINING its source
scalar (idx = pos//64, placed so idx position c*128+p lands at grid slot
(p, c)).  The wanted scalar is then selected by an on-device one-hot mask,
is_equal(off[p,c], iota64) with off=pos%64 (255 for padding slots), times
a free-axis add-reduce.  The gather is DMA-queue-bandwidth bound: 4 SWDGE
queues x ~22.5B/ns x 256B/idx => ~0.74ms per 216k-slot pass per core,
REACHED only with >= ~8 gather calls in flight (gsel pool bufs=12; with
bufs=5 only ~2.5 calls overlap and a pass takes 1.24ms).

Both layers share ONE idx/off table: the x table and the h2 table are both
laid out in padded-original node order (node n at row n//NPC*12544 +
n%NPC), so pos = pgid(src) works for both gathers.  x arrives as a 50KB
per-core slice and is AllGathered on device; h2_own (processing order) is
unpermuted into original order by a small 13-call on-device gather before
its AllGather.  The [P,C] column mask is derived on device from off==255,
and off ships as uint8 -- total ExternalInput traffic is ~0.8MB/core
(~6MB all-core) vs 24.4MB for the two-table variant.

Self-loops (rank 0 of every dst segment) are folded into denom/zt directly
from x_own/h2_own, shrinking every block's column width by 1 (C 1776 ->
1688).  Device exec ~2.0ms/core: 2 x 0.74ms edge-gather passes + ~0.15ms
small gathers/collectives + ~0.4ms vector work.

(Walrus quirks: >2 sem waits per instruction are split onto injected NoOps;
drains likewise. tile's DMASW lane rotation must match the dma_gather
queue_num rotation, so NUM_SWDGE_GLOBAL_SEMS == NQ and dma_gather is the
only Pool-engine DMA in the program.)
"""
import sys
sys.path.insert(0, "/opt/trn_rl_repo")
import re
import numpy as np
import ml_dtypes
import concourse.bass as bass
import concourse.mybir as mybir
import concourse.tile as tile
from concourse.library_config import mlp
from concourse.library_overlay import lower_extended_insts
from concourse.bass_utils import run_bass_kernel_spmd
from bass_rust import ScopedClock, VectorClock

N = 100000
NCORES = 8
NPC = N // NCORES          # nodes per core
P = 128
NBLK = (NPC + P - 1) // P  # 98
ROWS = NBLK * P            # 12544
NEG_SLOPE = 0.2
EPS = 1e-16
BIG_NEG = -1.0e30
USE_ACT_LRELU = False
CHUNK = 64                 # gather element width (f32) = 256B
GC = 8                     # grid columns per dma_gather call (1024 idxs)
NQ = 4                     # SWDGE queues
NCHT = 1568                # table chunks = 100352/64
SL = ROWS // CHUNK         # 196 chunks per per-core table slice
CU = 104                   # unpermute-gather grid cols (13 calls x GC)
import os
REP_GATHER = int(os.environ.get("REP_GATHER", "1"))   # timing-slope knob
GSEL_BUFS = int(os.environ.get("GSEL_BUFS", "12"))    # in-flight gather tiles

F32 = mybir.dt.float32
BF16 = mybir.dt.bfloat16
I16 = mybir.dt.int16
U8 = mybir.dt.uint8
AT = mybir.AluOpType
AF = mybir.ActivationFunctionType


# ---------------------------------------------------------------------------
# Tile tail-drain workaround: walrus TPB_CTRL codegen rejects a Drain with
# more than two sem waits; emit one NOP-wait per proc first.
def _split_drain_and_barrier(self, tick_clock, wait_clock):
    gc = tick_clock.global_clock
    ticks = [int(x) for x in re.findall(r"\d+", repr(gc))]
    for i, t in enumerate(ticks):
        if t <= 0:
            continue
        sub = VectorClock()
        sub.require_at_least(i, t)
        inst = self.nc.sync.nop()
        wait_clock.add_sem_waits(inst.ins, ScopedClock({None: sub}))
    self.nc.sync.drain()
    self.nc.all_engine_barrier()
    popped = self.nc._tile_sem_poison_stack.pop()
    assert popped is self._sem_poison
    self.nc.clear_and_free_semaphores(list(self.sems.allocated().values()))
    self.nc.all_engine_barrier()


tile.TileContext._drain_and_barrier = _split_drain_and_barrier

import concourse.tile_sem_assignment as _tsa
_tsa.NUM_SWDGE_GLOBAL_SEMS = NQ
_tsa.NUM_HWDGE_SEMS = 1

# Walrus encodes at most ~2 sem waits per instruction; split any excess onto
# injected same-engine NoOps in the BIR JSON right before compilation.
import json as _json
from concourse import bass2jax as _b2j


def _split_waits_json(bir, max_keep=1):
    d = _json.loads(bir)
    ctr = [0]

    def fix_block(blk):
        out = []
        for inst in blk.get("instructions", []):
            si = inst.get("sync_info")
            waits = (si or {}).get("on_wait") or []
            if len(waits) > max_keep and inst.get("opcode") != "NoOp":
                keep = waits[-max_keep:]
                for w in waits[:-max_keep]:
                    ctr[0] += 1
                    out.append({"debug": inst.get("debug", 0),
                                "engine": inst["engine"], "ins": [],
                                "outs": [], "name": f"I-wsp{ctr[0]}",
                                "opcode": "NoOp",
                                "sync_info": {"on_update": [], "on_wait": [w]}})
                si["on_wait"] = keep
            out.append(inst)
        blk["instructions"] = out
        for sb in blk.get("blocks", []):
            fix_block(sb)

    for fn in d["functions"]:
        for blk in fn["blocks"]:
            fix_block(blk)
    return _json.dumps(d).encode()


if not getattr(_b2j, "_wsplit_patched", False):
    _orig_cbk = _b2j.compile_bir_kernel

    def _cbk(bir, *a, **k):
        return _orig_cbk(_split_waits_json(bir), *a, **k)

    _b2j.compile_bir_kernel = _cbk
    _b2j._wsplit_patched = True


# ---------------------------------------------------------------------------
# CPU-side structural prep (graph topology only, no float math)

def _prep(edge_index):
    src = np.asarray(edge_index[0], dtype=np.int64)
    dst = np.asarray(edge_index[1], dtype=np.int64)
    loop = np.arange(N, dtype=np.int64)
    src = np.concatenate([loop, src])
    dst = np.concatenate([loop, dst])

    deg = np.bincount(dst, minlength=N)

    perms = []        # per core: processing order (local node ids 0..NPC-1)
    blk_deg = np.zeros((NCORES, NBLK), dtype=np.int64)
    for k in range(NCORES):
        dk = deg[k * NPC:(k + 1) * NPC]
        order = np.argsort(-dk, kind="stable").astype(np.int64)
        perms.append(order)
        dks = np.concatenate([dk[order], np.zeros(ROWS - NPC, np.int64)])
        blk_deg[k] = dks.reshape(NBLK, P).max(1)

    # Uniform group structure across cores: W per block = max over cores,
    # then greedily merge consecutive blocks (pad to group max) keeping the
    # added padding under ~8% and the per-group volume bounded.
    # Self-loops (rank 0 of every node, value available on-core) are folded
    # into the segment sums directly, so the grid only holds ranks >= 1.
    wblk = blk_deg.max(0) - 1      # [NBLK], non-increasing
    groups = []                    # list of (start_blk, end_blk, W)
    g0 = 0
    waste = 0.0
    real = 1.0
    for b in range(1, NBLK + 1):
        merge = False
        if b < NBLK:
            new_waste = waste + (wblk[g0] - wblk[b])
            new_real = real + wblk[b]
            vol = (b + 1 - g0) * max(wblk[g0], 1)
            if new_waste <= 0.08 * new_real and vol <= 256:
                merge = True
        if merge:
            waste, real = new_waste, new_real
            continue
        groups.append((g0, b, int(max(wblk[g0], 1))))
        if b < NBLK:
            g0 = b
            waste = 0.0
            real = float(wblk[b])
    col_off = np.zeros(NBLK, dtype=np.int64)
    C = 0
    for (a, b, w) in groups:
        for blk in range(a, b):
            col_off[blk] = C
            C += w
    if C % GC:
        C += GC - C % GC        # gather-call granularity

    # Both gather tables (x and h2) live in padded-original node order:
    # node n at row (n // NPC) * ROWS + (n % NPC).  One idx/off table
    # serves both layers.
    pos = np.zeros((NCORES, P, C), dtype=np.int64)
    valid = np.zeros((NCORES, P, C), dtype=bool)

    order = np.argsort(dst, kind="stable")
    src_s = src[order]
    dst_s = dst[order]
    starts = np.searchsorted(dst_s, np.arange(N + 1))

    posu = np.zeros((NCORES, P, CU), dtype=np.int64)   # unpermute gather
    validu = np.zeros((NCORES, P, CU), dtype=bool)

    for k in range(NCORES):
        inv_k = np.zeros(NPC, dtype=np.int64)
        inv_k[perms[k]] = np.arange(NPC)
        base = k * NPC
        lo, hi = starts[base], starts[base + NPC]
        d_loc = dst_s[lo:hi] - base                  # local dst id
        i_proc = inv_k[d_loc]                        # processing index
        blk = i_proc // P
        p = i_proc - blk * P
        seg_start = starts[d_loc + base] - lo        # rank within segment
        rank = np.arange(hi - lo) - seg_start
        nsl = rank >= 1                              # rank 0 = self-loop
        c = col_off[blk[nsl]] + rank[nsl] - 1
        p = p[nsl]
        s = src_s[lo:hi][nsl]
        pos[k, p, c] = (s // NPC) * ROWS + (s % NPC)
        valid[k, p, c] = True

        # unpermute: slot (p, cu) holds orig-local id l = cu*128+p, value
        # read from the proc-order scratch at flat position inv_k[l]
        l = np.arange(NPC)
        posu[k, l % P, l // P] = inv_k[l]
        validu[k, l % P, l // P] = True

    return {
        "groups": groups, "C": C, "perms": perms,
        "pos": pos, "valid": valid, "posu": posu, "validu": validu,
    }


def _mk_gather_inputs(pos, valid, C):
    """idx tile [16, ncalls*64] int16 + within-chunk offsets [P, C] uint8
    (invalid slots get 255 so the on-device iota-eq mask comes out 0)."""
    ncalls = C // GC
    chunk = (pos // CHUNK).astype(np.int16)          # [P, C]
    off = np.where(valid, pos % CHUNK, 255).astype(np.uint8)
    arr = chunk.reshape(P, ncalls, GC).transpose(1, 2, 0).reshape(
        ncalls, GC * P)                              # [g, j] j=c_local*128+p
    w = arr.reshape(ncalls, (GC * P) // 16, 16).transpose(0, 2, 1)
    t16 = np.ascontiguousarray(
        w.transpose(1, 0, 2).reshape(16, ncalls * 64).astype(np.int16))
    return t16, np.ascontiguousarray(off)


# ---------------------------------------------------------------------------
# Bass program (identical for all cores; per-core data differs)

def _build(C, groups):
    nc = bass.Bass("TRN2", target_bir_lowering=False, debug=False,
                   num_devices=NCORES, num_swdge_queues=NQ,
                   dynamic_dma_scratch_size=65536)
    ncalls = C // GC
    xsl = nc.dram_tensor("xsl", [SL, CHUNK], F32, kind="ExternalInput").ap()
    x_own = nc.dram_tensor("x_own", [P, NBLK], F32, kind="ExternalInput").ap()
    idxa = nc.dram_tensor("idxa", [16, ncalls * 64], I16,
                          kind="ExternalInput").ap()
    offa = nc.dram_tensor("offa", [P, C], U8, kind="ExternalInput").ap()
    idxu = nc.dram_tensor("idxu", [16, (CU // GC) * 64], I16,
                          kind="ExternalInput").ap()
    offu = nc.dram_tensor("offu", [P, CU], U8, kind="ExternalInput").ap()
    iota64 = nc.dram_tensor("iota64", [1, 64], F32, kind="ExternalInput").ap()
    w1 = nc.dram_tensor("w1", [1, 128], F32, kind="ExternalInput").ap()
    as1 = nc.dram_tensor("as1", [1, 128], F32, kind="ExternalInput").ap()
    ad1 = nc.dram_tensor("ad1", [1, 128], F32, kind="ExternalInput").ap()
    b1 = nc.dram_tensor("b1", [1, 128], F32, kind="ExternalInput").ap()
    w2 = nc.dram_tensor("w2", [1, 128], F32, kind="ExternalInput").ap()
    sc2 = nc.dram_tensor("sc2", [1, 8], F32, kind="ExternalInput").ap()
    # sc2 row: [att_src2, att_dst2, b2, 0 | iota4]

    out_d = nc.dram_tensor("out", [P, NBLK], F32, kind="ExternalOutput").ap()
    x_int = nc.dram_tensor("x_int", [SL, CHUNK], F32, kind="Internal").ap()
    xt_full = nc.dram_tensor("xt_full", [NCHT, CHUNK], F32,
                             kind="Internal", addr_space="Shared").ap()
    h2scr = nc.dram_tensor("h2scr", [SL, CHUNK], F32, kind="Internal").ap()
    h2sl = nc.dram_tensor("h2sl", [SL, CHUNK], F32, kind="Internal").ap()
    h2t_full = nc.dram_tensor("h2t_full", [NCHT, CHUNK], F32,
                              kind="Internal", addr_space="Shared").ap()

    with tile.TileContext(nc, num_cores=NCORES) as tc:
        _body(nc, tc, C, groups, xsl, x_own, idxa, offa, idxu, offu, iota64,
              w1, as1, ad1, b1, w2, sc2, out_d,
              x_int, xt_full, h2scr, h2sl, h2t_full)
    lower_extended_insts(nc)
    return nc


def _gather_v2(nc, tc, C, table_d, idx_t, off_t, iota_t, xs, nreg, qctr):
    """xs[p, c] = table.flat[pos[p, c]] via chunked dma_gather + mask-select.

    Per call: 1024 chunk-indices -> [128, GC, 64] f32; the one-hot select
    mask is built on-device as is_equal(off[p,c], iota[k]) (invalid slots
    carry off=255 so they select nothing), then mask-mult + free-axis add
    reduce pick out the wanted scalar per slot."""
    ncalls = C // GC
    with tc.tile_pool(name="gsel", bufs=GSEL_BUFS) as gp:
        for g in range(ncalls):
            ch = gp.tile([P, GC * CHUNK], F32, tag="ch")
            ch3 = ch[:].rearrange("p (g k) -> p g k", k=CHUNK)
            nc.gpsimd.dma_gather(
                ch3, table_d, idx_t[:, g * 64:(g + 1) * 64],
                GC * P, nreg, CHUNK, queue_num=qctr[0] % NQ)
            qctr[0] += 1
            m = gp.tile([P, GC * CHUNK], F32, tag="m")
            nc.vector.tensor_tensor(
                out=m[:].rearrange("p (g k) -> p g k", k=CHUNK),
                in0=off_t[:, g * GC:(g + 1) * GC].rearrange(
                    "p g -> p g ()").to_broadcast([P, GC, CHUNK]),
                in1=iota_t.rearrange("p k -> p () k").to_broadcast(
                    [P, GC, CHUNK]),
                op=AT.is_equal)
            nc.vector.tensor_tensor(out=ch[:], in0=ch[:], in1=m[:],
                                    op=AT.mult)
            nc.vector.tensor_reduce(
                out=xs[:, g * GC:(g + 1) * GC],
                in_=ch3, axis=mybir.AxisListType.X, op=AT.add)


def _body(nc, tc, C, groups, xsl_d, x_own_d, idxa_d, offa_d, idxu_d, offu_d,
          iota64_d, w1_d, as1_d, ad1_d, b1_d, w2_d, sc2_d, out_d,
          x_int, xt_full, h2scr, h2sl, h2t_full):
    import contextlib
    ctx = contextlib.ExitStack()
    H = 8
    ncalls = C // GC
    with ctx:
        nc.gpsimd.load_library(mlp)
        nreg = nc.gpsimd.to_reg(GC * P)
        qctr = [0]
        const = ctx.enter_context(tc.tile_pool(name="const", bufs=1))
        group_c0 = {}
        _c = 0
        for (ga, gb, gw) in groups:
            group_c0[ga] = _c
            _c += gw * (gb - ga)

        # ---- persistent loads
        idx_t = const.tile([P, ncalls * 64], I16)
        for k in range(8):
            nc.sync.dma_start(idx_t[16 * k:16 * (k + 1), :], idxa_d[:])
        off8 = const.tile([P, C], U8)
        nc.sync.dma_start(off8[:], offa_d[:])
        off_t = const.tile([P, C], F32)
        nc.vector.tensor_copy(out=off_t[:], in_=off8[:])
        idxu_t = const.tile([P, (CU // GC) * 64], I16)
        for k in range(8):
            nc.sync.dma_start(idxu_t[16 * k:16 * (k + 1), :], idxu_d[:])
        offu8 = const.tile([P, CU], U8)
        nc.sync.dma_start(offu8[:], offu_d[:])
        offu_t = const.tile([P, CU], F32)
        nc.vector.tensor_copy(out=offu_t[:], in_=offu8[:])
        x_own = const.tile([P, NBLK], F32)
        nc.sync.dma_start(x_own[:], x_own_d[:])

        # mask: BIG_NEG where the slot is padding (off == 255)
        bigneg = const.tile([P, 1], F32)
        nc.vector.memset(bigneg[:], BIG_NEG)
        mneg = const.tile([P, C], F32)
        nc.vector.scalar_tensor_tensor(
            out=mneg[:], in0=off_t[:], scalar=255.0,
            in1=bigneg[:].to_broadcast([P, C]),
            op0=AT.is_equal, op1=AT.mult)

        # ---- stage x slice to Internal and AllGather the x table
        xstg = const.tile([P, NBLK], F32)
        nc.sync.dma_start(
            xstg[:],
            xsl_d[:].rearrange("r c -> (r c)").rearrange("(b p) -> p b", p=P))
        nc.sync.dma_start(
            x_int[:].rearrange("r c -> (r c)").rearrange("(b p) -> p b", p=P),
            xstg[:])
        nc.gpsimd.collective_compute(
            "AllGather", AT.bypass,
            replica_groups=[list(range(NCORES))],
            ins=[x_int[:]], outs=[xt_full[:]])

        # ---- params: one row, then broadcast via ones-matmul
        # 0:128 w1 | 128:256 as1 | 256:384 ad1 | 384:512 b1 | 512:640 w2
        # 640:648 sc2 (att_src2, att_dst2, b2, w2sum | iota4)
        # 648:656 s1 | 656:664 d1 | 664:728 iota64
        prow = const.tile([1, 728], F32)
        nc.sync.dma_start(prow[:, 0:128], w1_d[:])
        nc.sync.dma_start(prow[:, 128:256], as1_d[:])
        nc.sync.dma_start(prow[:, 256:384], ad1_d[:])
        nc.sync.dma_start(prow[:, 384:512], b1_d[:])
        nc.sync.dma_start(prow[:, 512:640], w2_d[:])
        nc.sync.dma_start(prow[:, 640:648], sc2_d[:])
        nc.sync.dma_start(prow[:, 664:728], iota64_d[:])
        tmp = const.tile([1, 256], F32)
        nc.vector.tensor_tensor(out=tmp[:, 0:128], in0=prow[:, 0:128],
                                in1=prow[:, 128:256], op=AT.mult)
        nc.vector.tensor_tensor(out=tmp[:, 128:256], in0=prow[:, 0:128],
                                in1=prow[:, 256:384], op=AT.mult)
        nc.vector.tensor_reduce(out=prow[:, 648:664],
                                in_=tmp[:].rearrange("a (h c) -> a h c", c=16),
                                axis=mybir.AxisListType.X, op=AT.add)
        nc.vector.tensor_reduce(out=prow[:, 643:644], in_=prow[:, 512:640],
                                axis=mybir.AxisListType.X, op=AT.add)

        ones = const.tile([1, P], F32)
        nc.vector.memset(ones[:], 1.0)
        # funnel prow through one DVE copy so the matmul (whose load-weights
        # encoding has a tight sem-wait budget) depends on a single producer
        prow2 = const.tile([1, 728], F32)
        nc.vector.tensor_copy(out=prow2[:], in_=prow[:])
        psum = ctx.enter_context(tc.tile_pool(name="psum", bufs=2,
                                              space="PSUM"))
        pc = const.tile([P, 728], F32)
        for lo, hi in ((0, 512), (512, 728)):
            pcast = psum.tile([P, 512], F32, tag="pcast")
            nc.tensor.matmul(pcast[:, :hi - lo], lhsT=ones[:],
                             rhs=prow2[:, lo:hi], start=True, stop=True)
            nc.vector.tensor_copy(out=pc[:, lo:hi], in_=pcast[:, :hi - lo])
        W1t = pc[:, 0:128]
        B1t = pc[:, 384:512]
        W2t = pc[:, 512:640]
        s2c = pc[:, 640:641]
        d2c = pc[:, 641:642]
        b2c = pc[:, 642:643]
        w2sum = pc[:, 643:644]
        s1c = pc[:, 648:656]
        d1c = pc[:, 656:664]
        iota_t = pc[:, 664:728]

        # ---- gather x[src] (layer 1)
        xs = const.tile([P, C], F32)
        for _ in range(REP_GATHER):
            _gather_v2(nc, tc, C, xt_full, idx_t, off_t, iota_t, xs,
                       nreg, qctr)

        # adst[p, b, h] = x_own[p, b] * d1[h]
        adst = const.tile([P, NBLK * H], F32)
        nc.vector.tensor_tensor(
            out=adst[:].rearrange("p (b h) -> p b h", h=H),
            in0=x_own[:].rearrange("p b -> p b ()").to_broadcast([P, NBLK, H]),
            in1=d1c.rearrange("p h -> p () h").to_broadcast([P, NBLK, H]),
            op=AT.mult)

        denom = const.tile([P, NBLK * H], F32)
        zt = const.tile([P, NBLK * H], F32)

        # ---- layer-1 main, one run per block-group
        with tc.tile_pool(name="work", bufs=2) as work:
            _layer1_main(nc, C, groups, group_c0, work, xs, mneg, adst, s1c,
                         denom, zt)

        # ---- fold the self-loop edge (src == dst) into denom/zt directly:
        # e_self = lrelu((s1+d1)*x_own); denom += exp(e_self);
        # zt += exp(e_self)*x_own.  (Pad rows add exp(0)=1 to dead nodes.)
        sl1 = const.tile([P, H], F32)
        nc.vector.tensor_tensor(out=sl1[:], in0=s1c, in1=d1c, op=AT.add)
        us = const.tile([P, NBLK * H], F32)
        nc.vector.tensor_tensor(
            out=us[:].rearrange("p (b h) -> p b h", h=H),
            in0=x_own[:].rearrange("p b -> p b ()").to_broadcast([P, NBLK, H]),
            in1=sl1[:].rearrange("p h -> p () h").to_broadcast([P, NBLK, H]),
            op=AT.mult)
        nc.vector.scalar_tensor_tensor(out=us[:], in0=us[:], scalar=NEG_SLOPE,
                                       in1=us[:], op0=AT.mult, op1=AT.max)
        us2 = const.tile([P, NBLK * H], F32)
        nc.scalar.activation(out=us2[:], in_=us[:], func=AF.Exp)
        nc.vector.tensor_tensor(out=denom[:], in0=denom[:], in1=us2[:],
                                op=AT.add)
        nc.vector.tensor_tensor(
            out=us2[:].rearrange("p (b h) -> p b h", h=H),
            in0=us2[:].rearrange("p (b h) -> p b h", h=H),
            in1=x_own[:].rearrange("p b -> p b ()").to_broadcast([P, NBLK, H]),
            op=AT.mult)
        nc.vector.tensor_tensor(out=zt[:], in0=zt[:], in1=us2[:], op=AT.add)

        # ---- layer-1 epilogue -> h2_own [P, NBLK]
        r = const.tile([P, NBLK * H], F32)
        nc.vector.tensor_scalar(out=r[:], in0=denom[:], scalar1=float(EPS),
                                scalar2=None, op0=AT.add)
        nc.vector.reciprocal(out=r[:], in_=r[:])
        nc.vector.tensor_tensor(out=r[:], in0=r[:], in1=zt[:], op=AT.mult)

        h2_own = const.tile([P, NBLK], F32)
        with tc.tile_pool(name="ep", bufs=2) as ep:
            _epilogue(nc, ep, r, W1t, B1t, W2t, h2_own)
        nc.vector.tensor_scalar(out=h2_own[:], in0=h2_own[:], scalar1=w2sum,
                                scalar2=None, op0=AT.subtract)
        _rest(nc, tc, C, groups, group_c0, const, mneg, h2_own, idx_t,
              idxu_t, offu_t, off_t, iota_t, s2c, d2c, b2c, out_d,
              h2scr, h2sl, h2t_full, nreg, qctr)


def _layer1_main(nc, C, groups, group_c0, work, xs, mneg, adst, s1c,
                 denom, zt):
        H = 8
        for (a, b, w) in groups:
            nb = b - a
            c0 = group_c0[a]
            V = nb * H * w
            xs_g = xs[:, c0:c0 + nb * w].rearrange("p (n w) -> p n () w", w=w)
            mn_g = mneg[:, c0:c0 + nb * w].rearrange("p (n w) -> p n () w", w=w)
            ad_g = adst[:, a * H:b * H].rearrange("p (n h) -> p n h ()", h=H)
            s1_g = s1c.rearrange("p h -> p () h ()")

            u = work.tile([P, V], F32, tag="u")
            u4 = u[:].rearrange("p (n h w) -> p n h w", h=H, w=w)
            nc.vector.tensor_tensor(out=u4, in0=xs_g.to_broadcast([P, nb, H, w]),
                                    in1=s1_g.to_broadcast([P, nb, H, w]), op=AT.mult)
            u2 = work.tile([P, V], F32, tag="u2")
            u24 = u2[:].rearrange("p (n h w) -> p n h w", h=H, w=w)
            nc.vector.tensor_tensor(out=u24, in0=u4,
                                    in1=ad_g.to_broadcast([P, nb, H, w]), op=AT.add)
            nc.vector.tensor_tensor(out=u4, in0=u24,
                                    in1=mn_g.to_broadcast([P, nb, H, w]), op=AT.add)
            # leaky relu: max(0.2*v, v), then exp
            if USE_ACT_LRELU:
                nc.scalar.activation(out=u24, in_=u4, func=AF.Lrelu,
                                     alpha=NEG_SLOPE)
            else:
                nc.vector.scalar_tensor_tensor(out=u24, in0=u4, scalar=NEG_SLOPE,
                                               in1=u4, op0=AT.mult, op1=AT.max)
            ex = work.tile([P, V], F32, tag="ex")
            ex4 = ex[:].rearrange("p (n h w) -> p n h w", h=H, w=w)
            nc.scalar.activation(out=ex4, in_=u24, func=AF.Exp)
            nc.vector.tensor_reduce(
                out=denom[:, a * H:b * H].rearrange("p (n h) -> p n h", h=H),
                in_=ex4, axis=mybir.AxisListType.X, op=AT.add)
            nc.vector.tensor_tensor(out=u4, in0=ex4,
                                    in1=xs_g.to_broadcast([P, nb, H, w]), op=AT.mult)
            nc.vector.tensor_reduce(
                out=zt[:, a * H:b * H].rearrange("p (n h) -> p n h", h=H),
                in_=u4, axis=mybir.AxisListType.X, op=AT.add)

def _epilogue(nc, ep, r, W1t, B1t, W2t, h2_own):
        H = 8
        EPB = 14
        for a in range(0, NBLK, EPB):
            b = min(a + EPB, NBLK)
            nb = b - a
            V = nb * 128
            v = ep.tile([P, EPB * 128], F32, tag="v")
            v4 = v[:, :V].rearrange("p (n h c) -> p n h c", h=H, c=16)
            r_g = r[:, a * H:b * H].rearrange("p (n h) -> p n h ()", h=H)
            w1_g = W1t.rearrange("p (h c) -> p () h c", c=16)
            b1_g = B1t.rearrange("p (h c) -> p () h c", c=16)
            nc.vector.tensor_tensor(out=v4, in0=r_g.to_broadcast([P, nb, H, 16]),
                                    in1=w1_g.to_broadcast([P, nb, H, 16]),
                                    op=AT.mult)
            v2 = ep.tile([P, EPB * 128], F32, tag="v2")
            nc.vector.tensor_tensor(
                out=v2[:, :V].rearrange("p (n h c) -> p n h c", h=H, c=16),
                in0=v4, in1=b1_g.to_broadcast([P, nb, H, 16]), op=AT.add)
            # h1' = max(v,0) + min(exp(v),1);  elu(v) = h1' - 1
            ev = ep.tile([P, EPB * 128], F32, tag="ev")
            nc.scalar.activation(out=ev[:, :V], in_=v2[:, :V], func=AF.Exp)
            nc.vector.tensor_scalar(out=ev[:, :V], in0=ev[:, :V], scalar1=1.0,
                                    scalar2=None, op0=AT.min)
            nc.vector.tensor_scalar(out=v2[:, :V], in0=v2[:, :V], scalar1=0.0,
                                    scalar2=None, op0=AT.max)
            nc.vector.tensor_tensor(out=v2[:, :V], in0=v2[:, :V], in1=ev[:, :V],
                                    op=AT.add)
            # h2 = sum h1'*W2 - W2sum  (the elu -1 folded into W2sum)
            w2_g = W2t.rearrange("p (h c) -> p () (h c)", c=16)
            nc.vector.tensor_tensor(
                out=v4, in0=v2[:, :V].rearrange("p (n f) -> p n f", f=128),
                in1=w2_g.to_broadcast([P, nb, 128]), op=AT.mult)
            nc.vector.tensor_reduce(
                out=h2_own[:, a:b], in_=v4.rearrange("p n h c -> p n (h c)"),
                axis=mybir.AxisListType.X, op=AT.add)
def _rest(nc, tc, C, groups, group_c0, const, mneg, h2_own, idx_t,
          idxu_t, offu_t, off_t, iota_t, s2c, d2c, b2c, out_d,
          h2scr, h2sl, h2t_full, nreg, qctr):
        # ---- unpermute h2_own (processing order) into original node order:
        # store to scratch at flat position b*128+p (= processing index),
        # gather back at inv[l], store the orig-order slice, AllGather.
        nc.sync.dma_start(
            h2scr[:].rearrange("r c -> (r c)").rearrange(
                "(b p) -> p b", p=P),
            h2_own[:])
        h2o = const.tile([P, CU], F32)
        _gather_v2(nc, tc, CU, h2scr, idxu_t, offu_t, iota_t, h2o,
                   nreg, qctr)
        nc.sync.dma_start(
            h2sl[:].rearrange("r c -> (r c)").rearrange(
                "(c2 p) -> p c2", p=P),
            h2o[:, :NBLK])
        nc.gpsimd.collective_compute(
            "AllGather", AT.bypass,
            replica_groups=[list(range(NCORES))],
            ins=[h2sl[:]], outs=[h2t_full[:]])

        # ---- layer 2: same idx/off tables as layer 1 (same table layout)
        h2s = const.tile([P, C], F32)
        for _ in range(REP_GATHER):
            _gather_v2(nc, tc, C, h2t_full, idx_t, off_t, iota_t, h2s,
                       nreg, qctr)

        adst2 = const.tile([P, NBLK], F32)
        nc.vector.tensor_scalar(out=adst2[:], in0=h2_own[:], scalar1=d2c,
                                scalar2=None, op0=AT.mult)

        den2 = const.tile([P, NBLK], F32)
        z2 = const.tile([P, NBLK], F32)
        with tc.tile_pool(name="work2", bufs=2) as work:
            _layer2_main(nc, groups, group_c0, work, h2s, mneg, adst2, s2c,
                         den2, z2)

        # ---- self-loop fold, layer 2 (scalar per node)
        sl2 = const.tile([P, 1], F32)
        nc.vector.tensor_tensor(out=sl2[:], in0=s2c, in1=d2c, op=AT.add)
        u2s = const.tile([P, NBLK], F32)
        nc.vector.tensor_scalar(out=u2s[:], in0=h2_own[:], scalar1=sl2,
                                scalar2=None, op0=AT.mult)
        nc.vector.scalar_tensor_tensor(out=u2s[:], in0=u2s[:],
                                       scalar=NEG_SLOPE, in1=u2s[:],
                                       op0=AT.mult, op1=AT.max)
        u2e = const.tile([P, NBLK], F32)
        nc.scalar.activation(out=u2e[:], in_=u2s[:], func=AF.Exp)
        nc.vector.tensor_tensor(out=den2[:], in0=den2[:], in1=u2e[:],
                                op=AT.add)
        nc.vector.tensor_tensor(out=u2e[:], in0=u2e[:], in1=h2_own[:],
                                op=AT.mult)
        nc.vector.tensor_tensor(out=z2[:], in0=z2[:], in1=u2e[:], op=AT.add)
        _output(nc, den2, z2, b2c, out_d)


def _layer2_main(nc, groups, group_c0, work, h2s, mneg, adst2, s2c, den2, z2):
        for (a, b, w) in groups:
            nb = b - a
            c0 = group_c0[a]
            V = nb * w
            sl = slice(c0, c0 + V)
            h2s_g = h2s[:, sl].rearrange("p (n w) -> p n w", w=w)
            u = work.tile([P, V], F32, tag="u")
            u3 = u[:].rearrange("p (n w) -> p n w", w=w)
            nc.vector.scalar_tensor_tensor(
                out=u3, in0=h2s_g, scalar=s2c,
                in1=adst2[:, a:b].rearrange("p n -> p n ()").to_broadcast(
                    [P, nb, w]),
                op0=AT.mult, op1=AT.add)
            u2 = work.tile([P, V], F32, tag="u2")
            u23 = u2[:].rearrange("p (n w) -> p n w", w=w)
            nc.vector.tensor_tensor(
                out=u23, in0=u3,
                in1=mneg[:, sl].rearrange("p (n w) -> p n w", w=w), op=AT.add)
            if USE_ACT_LRELU:
                nc.scalar.activation(out=u3, in_=u23, func=AF.Lrelu,
                                     alpha=NEG_SLOPE)
            else:
                nc.vector.scalar_tensor_tensor(out=u3, in0=u23, scalar=NEG_SLOPE,
                                               in1=u23, op0=AT.mult, op1=AT.max)
            nc.scalar.activation(out=u23, in_=u3, func=AF.Exp)
            nc.vector.tensor_reduce(out=den2[:, a:b], in_=u23,
                                    axis=mybir.AxisListType.X, op=AT.add)
            nc.vector.tensor_tensor(out=u3, in0=u23, in1=h2s_g, op=AT.mult)
            nc.vector.tensor_reduce(out=z2[:, a:b], in_=u3,
                                    axis=mybir.AxisListType.X, op=AT.add)


def _output(nc, den2, z2, b2c, out_d):
        # ---- output
        nc.vector.tensor_scalar(out=den2[:], in0=den2[:], scalar1=float(EPS),
                                scalar2=None, op0=AT.add)
        nc.vector.reciprocal(out=den2[:], in_=den2[:])
        nc.vector.tensor_tensor(out=z2[:], in0=z2[:], in1=den2[:], op=AT.mult)
        nc.vector.tensor_scalar(out=z2[:], in0=z2[:], scalar1=b2c,
                                scalar2=None, op0=AT.add)
        nc.sync.dma_start(out_d[:], z2[:])


# ---------------------------------------------------------------------------

def kernel(**inputs):
    edge_index = np.asarray(inputs["edge_index"])
    prep = _prep(edge_index)
    C, groups, perms = prep["C"], prep["groups"], prep["perms"]

    x = np.asarray(inputs["x"], dtype=np.float32).reshape(-1)   # [N]

    flat = lambda a: np.ascontiguousarray(
        np.asarray(a, dtype=np.float32).reshape(1, -1))
    w1 = flat(inputs["W1"]); as1 = flat(inputs["att_src1"])
    ad1 = flat(inputs["att_dst1"]); b1 = flat(inputs["b1"])
    w2 = flat(inputs["W2"])
    sc2 = np.zeros((1, 8), np.float32)
    sc2[0, 0] = np.asarray(inputs["att_src2"]).reshape(-1)[0]
    sc2[0, 1] = np.asarray(inputs["att_dst2"]).reshape(-1)[0]
    sc2[0, 2] = np.asarray(inputs["b2"]).reshape(-1)[0]
    sc2[0, 4:8] = [0.0, 1.0, 2.0, 3.0]
    iota64 = np.arange(64, dtype=np.float32).reshape(1, 64)

    nc = _build(C, groups)

    in_maps = []
    for k in range(NCORES):
        xk_orig = x[k * NPC:(k + 1) * NPC]
        xsl = np.ascontiguousarray(
            np.concatenate([xk_orig, np.zeros(ROWS - NPC, np.float32)]
                           ).reshape(SL, CHUNK))
        xk = xk_orig[perms[k]]
        xk = np.concatenate([xk, np.zeros(ROWS - NPC, np.float32)])
        x_own = np.ascontiguousarray(xk.reshape(NBLK, P).T)
        ia, offa = _mk_gather_inputs(prep["pos"][k], prep["valid"][k], C)
        iu, offu = _mk_gather_inputs(prep["posu"][k], prep["validu"][k], CU)
        in_maps.append({
            "xsl": xsl, "x_own": x_own,
            "idxa": ia, "offa": offa, "idxu": iu, "offu": offu,
            "iota64": iota64,
            "w1": w1, "as1": as1, "ad1": ad1, "b1": b1, "w2": w2, "sc2": sc2,
        })

    res = run_bass_kernel_spmd(nc, in_maps, core_ids=list(range(NCORES)))

    out = np.zeros((N, 1), np.float32)
    for k in range(NCORES):
        o = res.results[k]["out"]                    # [P, NBLK]
        flat_o = o.T.reshape(-1)[:NPC]
        out[k * NPC:(k + 1) * NPC, 0][perms[k]] = flat_o
    return out

